# revision 6
# baseline (speedup 1.0000x reference)
"""Trainium2 Bass kernel for nn_AttentionLayer_Topic_60730837565900.

Architecture (8 NeuronCores, data-parallel over batch B=64 -> 8 batches/core):
  h  = (x @ W_in.T + in_b + te) * sqrt(0.5)         [B,T,E]
  s  = h @ (enc0+enc1)                              [B,T,S]
  a  = softmax(s, axis=-1)                          [B,T,S]   (output)
  c  = (a @ enc2) * sqrt(S)                         [B,T,E]
  o  = (c @ W_out.T + out_b + x) * sqrt(0.5)        [B,T,C]   (output)

On-chip layout is feature-major [feat, T] (host pre-transposes x/te). Scores are
computed per 128-row T-tile as [T,S] in PSUM, softmax is a fused
reduce_max -> exp(x-max) with accum_out row-sum, the normalized attention is
DMA'd out in natural [T,S] layout and PE-transposed (packed 4-per-PSUM-bank)
into [S,T] for the context matmul. h/scores matmuls run in float32r
(fp32-storage, ~bf16-speed, ~fp32-accuracy); ctx/out matmuls run in bf16.
Weight normalization, bias folding and scale folding are host-side prep.
"""
import sys
sys.path.insert(0, '/opt/trn_rl_repo')
import math
import numpy as np
import ml_dtypes

import concourse.bass as bass
import concourse.bacc as bacc
import concourse.mybir as mybir
import concourse.tile as tile
from concourse.bass_utils import run_bass_kernel_spmd

B, T, S, C, E = 64, 1024, 1024, 512, 512
NCORES = 8
BPC = B // NCORES
TCH = 512            # T processed per pipeline chunk
SQ = math.sqrt(0.5)

BF = mybir.dt.bfloat16
F32 = mybir.dt.float32
F32R = mybir.dt.float32r
NPBF = ml_dtypes.bfloat16

_CACHE = {}


def build(bpc=BPC):
    nc = bacc.Bacc("TRN2", target_bir_lowering=False, debug=False, num_devices=NCORES)

    xs_d = nc.dram_tensor("xs", [bpc, C, T], F32R, kind="ExternalInput")
    te_d = nc.dram_tensor("te", [bpc, E, T], F32, kind="ExternalInput")
    encs_d = nc.dram_tensor("encs", [bpc, E, S], F32R, kind="ExternalInput")
    enc2_d = nc.dram_tensor("enc2", [bpc, S, E], BF, kind="ExternalInput")
    win_d = nc.dram_tensor("win", [C, E], F32R, kind="ExternalInput")
    wout_d = nc.dram_tensor("wout", [E, C], BF, kind="ExternalInput")
    attn_d = nc.dram_tensor("attn", [bpc, T, S], BF, kind="ExternalOutput")
    out_d = nc.dram_tensor("outt", [bpc, C, T], BF, kind="ExternalOutput")

    with tile.TileContext(nc) as tc:
        from contextlib import ExitStack
        with ExitStack() as ctx:
            wpool = ctx.enter_context(tc.tile_pool(name="wpool", bufs=1))
            io = ctx.enter_context(tc.tile_pool(name="io", bufs=2))
            work = ctx.enter_context(tc.tile_pool(name="work", bufs=2))
            stats = ctx.enter_context(tc.tile_pool(name="stats", bufs=8))
            psum_mm = ctx.enter_context(tc.tile_pool(name="psum_mm", bufs=2, space="PSUM"))
            psum_sc = ctx.enter_context(tc.tile_pool(name="psum_sc", bufs=2, space="PSUM"))

            win = wpool.tile([128, 4, E], F32R)
            for c_t in range(4):
                nc.sync.dma_start(win[:, c_t, :], win_d[c_t * 128:(c_t + 1) * 128, :])
            wout = wpool.tile([128, 4, C], BF)
            for e_t in range(4):
                nc.sync.dma_start(wout[:, e_t, :], wout_d[e_t * 128:(e_t + 1) * 128, :])

            for b in range(bpc):
                encs = io.tile([128, 4, S], F32R, tag="encs")
                for e_t in range(4):
                    nc.sync.dma_start(encs[:, e_t, :], encs_d[b, e_t * 128:(e_t + 1) * 128, :])
                enc2 = io.tile([128, 8, E], BF, tag="enc2")
                for s_t in range(8):
                    nc.sync.dma_start(enc2[:, s_t, :], enc2_d[b, s_t * 128:(s_t + 1) * 128, :])
                out_sb = work.tile([128, 4, T], BF, tag="out_sb")

                for th in range(2):
                    t0 = th * TCH
                    xs = io.tile([128, 4, TCH], F32R, tag="xs")
                    for c_t in range(4):
                        nc.sync.dma_start(xs[:, c_t, :], xs_d[b, c_t * 128:(c_t + 1) * 128, t0:t0 + TCH])
                    te = io.tile([128, 4, TCH], F32, tag="te")
                    for e_t in range(4):
                        nc.sync.dma_start(te[:, e_t, :], te_d[b, e_t * 128:(e_t + 1) * 128, t0:t0 + TCH])

                    # ---- h = xs @ w_in + te  (f32r) ----
                    h = work.tile([128, 4, TCH], F32R, tag="h")
                    for e_t in range(4):
                        ps = psum_mm.tile([128, TCH], F32, tag="ps")
                        for c_t in range(4):
                            nc.tensor.matmul(ps[:], win[:, c_t, e_t * 128:(e_t + 1) * 128],
                                             xs[:, c_t, :], start=(c_t == 0), stop=(c_t == 3))
                        nc.vector.tensor_add(h[:, e_t, :], ps[:], te[:, e_t, :])

                    # ---- scores + softmax per 128-row T-tile ----
                    p = work.tile([128, 4, S], BF, tag="p")
                    for tt in range(4):
                        pssc = psum_sc.tile([128, S], F32, tag="pssc")
                        for s_ch in range(2):
                            for e_t in range(4):
                                nc.tensor.matmul(pssc[:, s_ch * 512:(s_ch + 1) * 512],
                                                 h[:, e_t, tt * 128:(tt + 1) * 128],
                                                 encs[:, e_t, s_ch * 512:(s_ch + 1) * 512],
                                                 start=(e_t == 0), stop=(e_t == 3))
                        nmax = stats.tile([128, 1], F32, tag="nmax")
                        nc.vector.reduce_max(nmax[:], pssc[:], axis=mybir.AxisListType.X, negate=True)
                        lsum = stats.tile([128, 1], F32, tag="lsum")
                        nc.scalar.activation(p[:, tt, :], pssc[:], mybir.ActivationFunctionType.Exp,
                                             bias=nmax[:], scale=1.0, accum_out=lsum[:])
                        rec = stats.tile([128, 1], F32, tag="rec")
                        nc.vector.reciprocal(rec[:], lsum[:])
                        nc.vector.tensor_scalar_mul(p[:, tt, :], p[:, tt, :], rec[:])
                        nc.sync.dma_start(attn_d[b, t0 + tt * 128:t0 + (tt + 1) * 128, :], p[:, tt, :])

                    # ---- transpose attn tiles into [S, T] layout (DMA xbar) ----
                    attnT = work.tile([128, 8, TCH], BF, tag="attnT")
                    for s_t in range(8):
                        for tt in range(4):
                            nc.scalar.dma_start(attnT[:, s_t, tt * 128:(tt + 1) * 128],
                                                p[:, tt, s_t * 128:(s_t + 1) * 128],
                                                transpose=True)

                    # ---- ctx = attnT.T @ enc2 (bf16; *32*sqrt(.5) folded into wout) ----
                    ctxt = work.tile([128, 4, TCH], BF, tag="ctxt")
                    for e_t in range(4):
                        ps = psum_mm.tile([128, TCH], F32, tag="ps")
                        for s_t in range(8):
                            nc.tensor.matmul(ps[:], enc2[:, s_t, e_t * 128:(e_t + 1) * 128],
                                             attnT[:, s_t, :], start=(s_t == 0), stop=(s_t == 7))
                        nc.scalar.copy(ctxt[:, e_t, :], ps[:])

                    # ---- out = ctx @ wout + xs ----
                    for c_t in range(4):
                        ps = psum_mm.tile([128, TCH], F32, tag="ps")
                        for e_t in range(4):
                            nc.tensor.matmul(ps[:], wout[:, e_t, c_t * 128:(c_t + 1) * 128],
                                             ctxt[:, e_t, :], start=(e_t == 0), stop=(e_t == 3))
                        nc.vector.tensor_add(out_sb[:, c_t, t0:t0 + TCH], ps[:], xs[:, c_t, :].bitcast(F32))

                for c_t in range(4):
                    nc.sync.dma_start(out_d[b, c_t * 128:(c_t + 1) * 128, :], out_sb[:, c_t, :])

    nc.compile()
    return nc


def get_nc():
    if "nc" not in _CACHE:
        _CACHE["nc"] = build()
    return _CACHE["nc"]


def prep_inputs(x, target_embedding, enc0, enc1, enc2, in_v, in_g, in_b, out_v, out_g, out_b):
    x = np.asarray(x, np.float32)
    te = np.asarray(target_embedding, np.float32)
    enc0 = np.asarray(enc0, np.float32)
    enc1 = np.asarray(enc1, np.float32)
    enc2 = np.asarray(enc2, np.float32)
    in_v = np.asarray(in_v, np.float32)
    in_g = np.asarray(in_g, np.float32)
    in_b = np.asarray(in_b, np.float32)
    out_v = np.asarray(out_v, np.float32)
    out_g = np.asarray(out_g, np.float32)
    out_b = np.asarray(out_b, np.float32)

    w_in = in_v * (in_g / np.linalg.norm(in_v, axis=1))[:, None]          # [E,C]
    w_out = out_v * (out_g / np.linalg.norm(out_v, axis=1))[:, None]      # [C,E]
    win_t = np.ascontiguousarray(w_in.T)                                  # [C,E] f32
    wout_t = np.ascontiguousarray((w_out * (32.0 * SQ)).T).astype(NPBF)   # [E,C] bf16
    xs_t = np.ascontiguousarray((x * SQ).transpose(0, 2, 1))              # [B,C,T] f32
    te_t = np.ascontiguousarray(
        (te * SQ + (SQ * in_b)[None, None, :]).transpose(0, 2, 1))        # [B,E,T] f32
    encs = enc0 + enc1                                                    # [B,E,S] f32
    enc2b = enc2.astype(NPBF)                                             # [B,S,E] bf16
    in_maps = []
    for i in range(NCORES):
        sl = slice(i * BPC, (i + 1) * BPC)
        in_maps.append({
            "xs": xs_t[sl],
            "te": te_t[sl],
            "encs": encs[sl],
            "enc2": enc2b[sl],
            "win": win_t,
            "wout": wout_t,
        })
    return in_maps, out_b


def postprocess(results, out_b):
    attn = np.concatenate([r["attn"] for r in results], axis=0).astype(np.float32)  # [B,T,S]
    out_ct = np.concatenate([r["outt"] for r in results], axis=0).astype(np.float32)  # [B,C,T]
    out = np.ascontiguousarray(out_ct.transpose(0, 2, 1)) + (SQ * out_b)[None, None, :]
    return out.astype(np.float32), attn


def kernel(**inputs):
    in_maps, out_b = prep_inputs(**inputs)
    nc = get_nc()
    res = run_bass_kernel_spmd(nc, in_maps, core_ids=list(range(NCORES)))
    return postprocess(res.results, out_b)
